# revision 10
# baseline (speedup 1.0000x reference)
"""GCN one-layer (Cora) on 8 TRN2 NeuronCores.

out = log_softmax(Ahat @ (x @ W) + b), Ahat = D^-1/2 (A + I) D^-1/2 (deg over dst).

Strategy (hardcoded for x[2708,143300] f32, W[143300,7], b[7], edge_index[2,10556]):
  - Row-shard x across 8 cores (340 rows each; core 7 overlaps core 6 by 12 rows
    so every core gets the same shape).  Host casts to fp16 and pre-transposes
    each shard into K-major tiles [1120, 128, 340] so the device GEMM needs no
    on-chip transpose and streams half the bytes (the problem is memory-bound).
  - Per core: h_shard^T = sum_t W[t].T @ xT[t]  (fp16 matmuls at full PE rate,
    fp32 PSUM accumulation; end-to-end output absmax error ~9e-4, dominated by
    the fp16 rounding of x, vs ~5e-4 for the 2.7x-slower fp32r variant).
  - PE-transpose h^T -> h, AllGather the tiny [340,7] shards across the chip.
  - Aggregate with a dense, host-built normalized adjacency (as lhsT tiles,
    fp32), add b via a K=1 matmul with a ones row, log_softmax on device.
  - Host stitches the 8 [340,7] outputs back to [2708,7].
"""

import numpy as np

import concourse.bass as bass
import concourse.mybir as mybir
import concourse.tile as tile
from concourse import bacc
from concourse import bass_utils
from concourse.masks import make_identity

N = 2708
K = 143300
OUT = 7
NCORES = 8
RPC = 340                 # rows per core (8*340 = 2720 >= 2708)
KT = 1120                 # 128-wide K chunks (1119 full + 68 remainder, zero padded)
KTAIL = K - 1119 * 128    # 68
CPD = 32                  # K chunks per DMA batch (2.78 MB fp16 per dma_start)
ST = 22                   # s-chunks for aggregation (2720 = 21*128 + 32)
SLAST = NCORES * RPC - 21 * 128  # 32
MTILES = [(0, 128), (128, 128), (256, RPC - 256)]  # out-row tiles per core

f32 = mybir.dt.float32
f16 = mybir.dt.float16

_STATE = {}


def _row_start(c):
    return c * RPC if c < NCORES - 1 else N - RPC


def _build_nc():
    nc = bacc.Bacc(None, target_bir_lowering=False)
    xt_d = nc.dram_tensor("xt", [KT, 128, RPC], f16, kind="ExternalInput")
    w_d = nc.dram_tensor("w", [128, KT * OUT], f16, kind="ExternalInput")
    a_d = nc.dram_tensor("at", [128, ST * RPC], f32, kind="ExternalInput")
    b_d = nc.dram_tensor("b", [1, OUT], f32, kind="ExternalInput")
    o_d = nc.dram_tensor("out", [RPC, OUT], f32, kind="ExternalOutput")

    with tile.TileContext(nc) as tc:
        with (
            tc.tile_pool(name="wpool", bufs=1) as wpool,
            tc.tile_pool(name="xpool", bufs=3) as xpool,
            tc.tile_pool(name="spool", bufs=1) as spool,
            tc.tile_pool(name="gps", bufs=1, space="PSUM") as gps,
            tc.tile_pool(name="ps2", bufs=2, space="PSUM") as ps2,
            tc.tile_pool(name="dram", bufs=1, space="DRAM") as dram,
        ):
            w_sb = wpool.tile([128, KT * OUT], f16)
            nc.sync.dma_start(w_sb[:], w_d[:])

            # ---- main GEMM: hT[7, RPC] += W[t].T @ xT[t] over 1120 K chunks.
            # Two PSUM accumulators (even/odd chunks) so consecutive matmuls
            # hit different banks — same-bank accumulation serializes each
            # fill behind the previous drain (~+128 cyc/matmul).
            hT_pa = gps.tile([OUT, RPC], f32, tag="pa")
            hT_pb = gps.tile([OUT, RPC], f32, tag="pb")
            accs = [hT_pa, hT_pb]
            nbatch = KT // CPD
            for bi in range(nbatch):
                xt_sb = xpool.tile([128, CPD, RPC], f16, tag="x")
                dma_eng = nc.sync if bi % 2 == 0 else nc.scalar
                dma_eng.dma_start(
                    xt_sb[:],
                    xt_d.ap()[bi * CPD:(bi + 1) * CPD].rearrange("t p r -> p t r"),
                )
                for ci in range(CPD):
                    t = bi * CPD + ci
                    nc.tensor.matmul(
                        accs[t % 2][:],
                        w_sb[:, t * OUT:(t + 1) * OUT],
                        xt_sb[:, ci, :],
                        start=(t < 2),
                        stop=(t >= KT - 2),
                    )

            # tail-only operands: loaded behind the x stream on the same queue
            a_sb = wpool.tile([128, ST * RPC], f32)
            nc.sync.dma_start(a_sb[:], a_d[:])
            b_sb = wpool.tile([1, OUT], f32)
            nc.sync.dma_start(b_sb[:], b_d[:])
            ident = wpool.tile([OUT, OUT], f32)
            make_identity(nc, ident[:])
            ones_sb = wpool.tile([1, 128], f32)
            nc.vector.memset(ones_sb[:], 1.0)

            hT_sb = spool.tile([OUT, RPC], f32)
            nc.vector.tensor_copy(hT_sb[:], hT_pa[:])
            nc.vector.tensor_add(hT_sb[:], hT_sb[:], hT_pb[:])

            # ---- transpose hT -> h shard, stage to DRAM, AllGather
            cc_in = dram.tile([RPC, OUT], f32)
            cc_out = dram.tile([NCORES * RPC, OUT], f32)
            for m0, mw in MTILES:
                tp_ps = ps2.tile([128, OUT], f32, tag="tp")
                nc.tensor.transpose(tp_ps[:mw, :], hT_sb[:, m0:m0 + mw], ident[:])
                hsh_sb = spool.tile([128, OUT], f32, tag="hsh")
                nc.vector.tensor_copy(hsh_sb[:mw, :], tp_ps[:mw, :])
                nc.sync.dma_start(cc_in[m0:m0 + mw, :], hsh_sb[:mw, :])
            nc.gpsimd.collective_compute(
                "AllGather",
                mybir.AluOpType.bypass,
                replica_groups=[list(range(NCORES))],
                ins=[cc_in.opt()],
                outs=[cc_out.opt()],
            )

            hall_sb = spool.tile([128, ST, OUT], f32)
            nc.sync.dma_start(
                hall_sb[:, :21, :],
                cc_out[:21 * 128, :].rearrange("(c p) j -> p c j", p=128),
            )
            nc.sync.dma_start(hall_sb[:SLAST, 21, :], cc_out[21 * 128:, :])

            # ---- aggregation + bias + log_softmax per 128-row out tile
            for m0, mw in MTILES:
                o_ps = ps2.tile([128, OUT], f32, tag="agg")
                for c in range(ST):
                    kc = 128 if c < 21 else SLAST
                    nc.tensor.matmul(
                        o_ps[:mw, :],
                        a_sb[:kc, c * RPC + m0:c * RPC + m0 + mw],
                        hall_sb[:kc, c, :],
                        start=(c == 0),
                        stop=False,
                    )
                nc.tensor.matmul(
                    o_ps[:mw, :], ones_sb[:1, :mw], b_sb[:1, :],
                    start=False, stop=True,
                )
                mx = spool.tile([128, 1], f32, tag="mx")
                nc.vector.reduce_max(mx[:mw], o_ps[:mw, :], axis=mybir.AxisListType.X)
                tt = spool.tile([128, OUT], f32, tag="tt")
                nc.vector.tensor_scalar_sub(tt[:mw, :], o_ps[:mw, :], mx[:mw])
                ee = spool.tile([128, OUT], f32, tag="ee")
                ss = spool.tile([128, 1], f32, tag="ss")
                nc.scalar.activation(
                    ee[:mw, :], tt[:mw, :], mybir.ActivationFunctionType.Exp,
                    accum_out=ss[:mw],
                )
                ls = spool.tile([128, 1], f32, tag="ls")
                nc.scalar.activation(
                    ls[:mw], ss[:mw], mybir.ActivationFunctionType.Ln,
                )
                oo = spool.tile([128, OUT], f32, tag="oo")
                nc.vector.tensor_scalar_sub(oo[:mw, :], tt[:mw, :], ls[:mw])
                nc.sync.dma_start(o_d.ap()[m0:m0 + mw, :], oo[:mw, :])

    nc.compile()
    return nc


def _prep_x_shard(x16, c):
    """fp16 [RPC, K] row shard -> K-major tiles [KT, 128, RPC] (zero-padded)."""
    r0 = _row_start(c)
    xc = x16[r0:r0 + RPC]
    out = np.zeros((KT, 128, RPC), dtype=np.float16)
    out[:1119] = xc[:, :1119 * 128].reshape(RPC, 1119, 128).transpose(1, 2, 0)
    out[1119, :KTAIL, :] = xc[:, 1119 * 128:].T
    return out


def _prep_adjacency(edge_index):
    """Dense Ahat^T tiles per core, laid out as [128, ST*RPC] lhsT slabs."""
    src = np.asarray(edge_index[0], dtype=np.int64)
    dst = np.asarray(edge_index[1], dtype=np.int64)
    deg = np.bincount(dst, minlength=N).astype(np.float64) + 1.0
    dinv = (1.0 / np.sqrt(deg)).astype(np.float32).astype(np.float64)
    A = np.zeros((N, N), dtype=np.float64)
    np.add.at(A, (dst, src), dinv[src] * dinv[dst])
    A[np.arange(N), np.arange(N)] += dinv * dinv
    A = A.astype(np.float32)

    # remap columns to the AllGather layout [NCORES*RPC]: slots 2380..2391 are
    # core 7's copies of rows 2368..2379 (already present) -> zeroed
    split = (NCORES - 1) * RPC                      # 2380
    dup = split - _row_start(NCORES - 1)            # 12
    A_ag = np.zeros((N, NCORES * RPC), dtype=np.float32)
    A_ag[:, :split] = A[:, :split]
    A_ag[:, split + dup:] = A[:, split:]

    slabs = []
    for c in range(NCORES):
        r0 = _row_start(c)
        at = np.zeros((ST * 128, RPC), dtype=np.float32)
        at[:NCORES * RPC] = A_ag[r0:r0 + RPC].T
        slabs.append(
            np.ascontiguousarray(
                at.reshape(ST, 128, RPC).transpose(1, 0, 2).reshape(128, ST * RPC)
            )
        )
    return slabs


def _prep_w(W):
    w_pad = np.zeros((KT * 128, OUT), dtype=np.float16)
    w_pad[:K] = W.astype(np.float16)
    return np.ascontiguousarray(
        w_pad.reshape(KT, 128, OUT).transpose(1, 0, 2).reshape(128, KT * OUT)
    )


def kernel(x, W, b, edge_index, _trace=False):
    x = np.asarray(x, dtype=np.float32)
    W = np.asarray(W, dtype=np.float32)
    b = np.asarray(b, dtype=np.float32)

    if "nc" not in _STATE:
        _STATE["nc"] = _build_nc()
    nc = _STATE["nc"]

    x16 = x.astype(np.float16)
    w_sb = _prep_w(W)
    a_slabs = _prep_adjacency(edge_index)
    b_in = np.ascontiguousarray(b.reshape(1, OUT))
    in_maps = [
        {"xt": _prep_x_shard(x16, c), "w": w_sb, "at": a_slabs[c], "b": b_in}
        for c in range(NCORES)
    ]

    res = bass_utils.run_bass_kernel_spmd(
        nc, in_maps, core_ids=list(range(NCORES)), trace=_trace,
    )
    _STATE["last_result"] = res

    out = np.empty((N, OUT), dtype=np.float32)
    for c in range(NCORES - 1):
        out[c * RPC:(c + 1) * RPC] = res.results[c]["out"]
    split = (NCORES - 1) * RPC
    out[split:] = res.results[NCORES - 1]["out"][split - _row_start(NCORES - 1):]
    return out


# revision 15
# speedup vs baseline: 1.0192x; 1.0192x over previous
"""GCN one-layer (Cora) on 8 TRN2 NeuronCores.

out = log_softmax(Ahat @ (x @ W) + b), Ahat = D^-1/2 (A + I) D^-1/2 (deg over dst).

Strategy (hardcoded for x[2708,143300] f32, W[143300,7], b[7], edge_index[2,10556]):
  - Row-shard x across 8 cores (340 rows each; core 7 overlaps core 6 by 12 rows
    so every core gets the same shape).  Host casts to fp16 and pre-transposes
    each shard into K-major tiles [1120, 128, 340] so the device GEMM needs no
    on-chip transpose and streams half the bytes (the problem is memory-bound).
  - Per core: h_shard^T = sum_t W[t].T @ xT[t]  (fp16 matmuls, fp32 PSUM).
    Each matmul only uses 7 of 128 PE columns, so 4 K-chunks are packed into
    the 4 column groups via tile_position and run CONCURRENTLY in the array —
    keeps TensorE well below the DMA stream even at the cold 1.2 GHz clock
    (HAM never warms for this shape).  End-to-end output absmax ~9e-4,
    dominated by fp16 rounding of x.
  - The 8 partition-group partials are reduced and transposed in one step by
    a matmul against a 0/1 selector matrix, then the [340,7] h shards are
    AllGathered across the chip.
  - Aggregate with a dense, host-built normalized adjacency (as lhsT tiles,
    fp32), add b via a K=1 matmul with a ones row, log_softmax on device.
  - Host stitches the 8 [340,7] outputs back to [2708,7].
"""

import numpy as np

import concourse.bass as bass
import concourse.mybir as mybir
import concourse.tile as tile
from concourse import bacc
from concourse import bass_utils

N = 2708
K = 143300
OUT = 7
NCORES = 8
RPC = 340                 # rows per core (8*340 = 2720 >= 2708)
KT = 1120                 # 128-wide K chunks (1119 full + 68 remainder, zero padded)
KTAIL = K - 1119 * 128    # 68
CPD = 32                  # K chunks per DMA batch (2.78 MB fp16 per dma_start)
ST = 22                   # s-chunks for aggregation (2720 = 21*128 + 32)
SLAST = NCORES * RPC - 21 * 128  # 32
MTILES = [(0, 128), (128, 128), (256, RPC - 256)]  # out-row tiles per core

f32 = mybir.dt.float32
f16 = mybir.dt.float16

_STATE = {}


def _row_start(c):
    return c * RPC if c < NCORES - 1 else N - RPC


def _build_nc():
    nc = bacc.Bacc(None, target_bir_lowering=False)
    xt_d = nc.dram_tensor("xt", [KT, 128, RPC], f16, kind="ExternalInput")
    w_d = nc.dram_tensor("w", [128, KT * OUT], f16, kind="ExternalInput")
    a_d = nc.dram_tensor("at", [128, ST * RPC], f32, kind="ExternalInput")
    b_d = nc.dram_tensor("b", [1, OUT], f32, kind="ExternalInput")
    m_d = nc.dram_tensor("msel", [128, OUT], f32, kind="ExternalInput")
    o_d = nc.dram_tensor("out", [RPC, OUT], f32, kind="ExternalOutput")

    with tile.TileContext(nc) as tc:
        with (
            tc.tile_pool(name="wpool", bufs=1) as wpool,
            tc.tile_pool(name="xpool", bufs=3) as xpool,
            tc.tile_pool(name="spool", bufs=1) as spool,
            tc.tile_pool(name="gps", bufs=1, space="PSUM") as gps,
            tc.tile_pool(name="ps2", bufs=2, space="PSUM") as ps2,
            tc.tile_pool(name="dram", bufs=1, space="DRAM") as dram,
        ):
            w_sb = wpool.tile([128, KT * OUT], f16)
            nc.sync.dma_start(w_sb[:], w_d[:])

            # ---- main GEMM: hT[7, RPC] += W[t].T @ xT[t] over 1120 K chunks,
            # 4 chunks packed into the 4 PE column groups (concurrent), two
            # PSUM banks alternating by group.
            ngroups = KT // 4
            hT_pa = gps.tile([128, RPC], f32, tag="pa")
            hT_pb = gps.tile([128, RPC], f32, tag="pb")
            banks = [hT_pa, hT_pb]
            nbatch = KT // CPD
            for bi in range(nbatch):
                xt_sb = xpool.tile([128, CPD, RPC], f16, tag="x")
                dma_eng = nc.sync if bi % 2 == 0 else nc.scalar
                dma_eng.dma_start(
                    xt_sb[:],
                    xt_d.ap()[bi * CPD:(bi + 1) * CPD].rearrange("t p r -> p t r"),
                )
                for ci in range(CPD):
                    t = bi * CPD + ci
                    g, c = t // 4, t % 4
                    nc.tensor.matmul(
                        banks[g % 2][32 * c:32 * c + OUT, :],
                        w_sb[:, t * OUT:(t + 1) * OUT],
                        xt_sb[:, ci, :],
                        tile_position=(0, 32 * c),
                        start=(g < 2),
                        stop=(g >= ngroups - 2),
                    )

            # tail-only operands: loaded behind the x stream on the same queue
            a_sb = wpool.tile([128, ST * RPC], f32)
            nc.sync.dma_start(a_sb[:], a_d[:])
            b_sb = wpool.tile([1, OUT], f32)
            nc.sync.dma_start(b_sb[:], b_d[:])
            m_sb = wpool.tile([128, OUT], f32)
            nc.sync.dma_start(m_sb[:], m_d[:])
            ones_sb = wpool.tile([1, 128], f32)
            nc.vector.memset(ones_sb[:], 1.0)

            # ---- reduce the 8 partition-group partials and transpose to
            # h[RPC, 7] in one step: h[r, j] = sum_bank sum_c psum[32c+j, r],
            # via matmuls against the 0/1 selector msel[32c+j, j] = 1.
            hTa = spool.tile([128, RPC], f32, tag="hta")
            nc.vector.tensor_copy(hTa[:], hT_pa[:])
            hTb = spool.tile([128, RPC], f32, tag="htb")
            nc.vector.tensor_copy(hTb[:], hT_pb[:])
            KSEL = 32 * 3 + OUT  # 103 partitions cover all four groups
            cc_in = dram.tile([RPC, OUT], f32)
            cc_out = dram.tile([NCORES * RPC, OUT], f32)
            for m0, mw in MTILES:
                h_ps = ps2.tile([128, OUT], f32, tag="tp")
                nc.tensor.matmul(h_ps[:mw, :], hTa[:KSEL, m0:m0 + mw],
                                 m_sb[:KSEL, :], start=True, stop=False)
                nc.tensor.matmul(h_ps[:mw, :], hTb[:KSEL, m0:m0 + mw],
                                 m_sb[:KSEL, :], start=False, stop=True)
                hsh_sb = spool.tile([128, OUT], f32, tag="hsh")
                nc.vector.tensor_copy(hsh_sb[:mw, :], h_ps[:mw, :])
                nc.sync.dma_start(cc_in[m0:m0 + mw, :], hsh_sb[:mw, :])
            nc.gpsimd.collective_compute(
                "AllGather",
                mybir.AluOpType.bypass,
                replica_groups=[list(range(NCORES))],
                ins=[cc_in.opt()],
                outs=[cc_out.opt()],
            )

            hall_sb = spool.tile([128, ST, OUT], f32)
            nc.sync.dma_start(
                hall_sb[:, :21, :],
                cc_out[:21 * 128, :].rearrange("(c p) j -> p c j", p=128),
            )
            nc.sync.dma_start(hall_sb[:SLAST, 21, :], cc_out[21 * 128:, :])

            # ---- aggregation + bias + log_softmax per 128-row out tile
            for m0, mw in MTILES:
                o_ps = ps2.tile([128, OUT], f32, tag="agg")
                for c in range(ST):
                    kc = 128 if c < 21 else SLAST
                    nc.tensor.matmul(
                        o_ps[:mw, :],
                        a_sb[:kc, c * RPC + m0:c * RPC + m0 + mw],
                        hall_sb[:kc, c, :],
                        start=(c == 0),
                        stop=False,
                    )
                nc.tensor.matmul(
                    o_ps[:mw, :], ones_sb[:1, :mw], b_sb[:1, :],
                    start=False, stop=True,
                )
                mx = spool.tile([128, 1], f32, tag="mx")
                nc.vector.reduce_max(mx[:mw], o_ps[:mw, :], axis=mybir.AxisListType.X)
                tt = spool.tile([128, OUT], f32, tag="tt")
                nc.vector.tensor_scalar_sub(tt[:mw, :], o_ps[:mw, :], mx[:mw])
                ee = spool.tile([128, OUT], f32, tag="ee")
                ss = spool.tile([128, 1], f32, tag="ss")
                nc.scalar.activation(
                    ee[:mw, :], tt[:mw, :], mybir.ActivationFunctionType.Exp,
                    accum_out=ss[:mw],
                )
                ls = spool.tile([128, 1], f32, tag="ls")
                nc.scalar.activation(
                    ls[:mw], ss[:mw], mybir.ActivationFunctionType.Ln,
                )
                oo = spool.tile([128, OUT], f32, tag="oo")
                nc.vector.tensor_scalar_sub(oo[:mw, :], tt[:mw, :], ls[:mw])
                nc.sync.dma_start(o_d.ap()[m0:m0 + mw, :], oo[:mw, :])

    nc.compile()
    return nc


def _prep_x_shard(x16, c):
    """fp16 [RPC, K] row shard -> K-major tiles [KT, 128, RPC] (zero-padded)."""
    r0 = _row_start(c)
    xc = x16[r0:r0 + RPC]
    out = np.zeros((KT, 128, RPC), dtype=np.float16)
    out[:1119] = xc[:, :1119 * 128].reshape(RPC, 1119, 128).transpose(1, 2, 0)
    out[1119, :KTAIL, :] = xc[:, 1119 * 128:].T
    return out


def _prep_adjacency(edge_index):
    """Dense Ahat^T tiles per core, laid out as [128, ST*RPC] lhsT slabs."""
    src = np.asarray(edge_index[0], dtype=np.int64)
    dst = np.asarray(edge_index[1], dtype=np.int64)
    deg = np.bincount(dst, minlength=N).astype(np.float64) + 1.0
    dinv = (1.0 / np.sqrt(deg)).astype(np.float32).astype(np.float64)
    A = np.zeros((N, N), dtype=np.float64)
    np.add.at(A, (dst, src), dinv[src] * dinv[dst])
    A[np.arange(N), np.arange(N)] += dinv * dinv
    A = A.astype(np.float32)

    # remap columns to the AllGather layout [NCORES*RPC]: slots 2380..2391 are
    # core 7's copies of rows 2368..2379 (already present) -> zeroed
    split = (NCORES - 1) * RPC                      # 2380
    dup = split - _row_start(NCORES - 1)            # 12
    A_ag = np.zeros((N, NCORES * RPC), dtype=np.float32)
    A_ag[:, :split] = A[:, :split]
    A_ag[:, split + dup:] = A[:, split:]

    slabs = []
    for c in range(NCORES):
        r0 = _row_start(c)
        at = np.zeros((ST * 128, RPC), dtype=np.float32)
        at[:NCORES * RPC] = A_ag[r0:r0 + RPC].T
        slabs.append(
            np.ascontiguousarray(
                at.reshape(ST, 128, RPC).transpose(1, 0, 2).reshape(128, ST * RPC)
            )
        )
    return slabs


def _prep_w(W):
    w_pad = np.zeros((KT * 128, OUT), dtype=np.float16)
    w_pad[:K] = W.astype(np.float16)
    return np.ascontiguousarray(
        w_pad.reshape(KT, 128, OUT).transpose(1, 0, 2).reshape(128, KT * OUT)
    )


def kernel(x, W, b, edge_index, _trace=False):
    x = np.asarray(x, dtype=np.float32)
    W = np.asarray(W, dtype=np.float32)
    b = np.asarray(b, dtype=np.float32)

    if "nc" not in _STATE:
        _STATE["nc"] = _build_nc()
    nc = _STATE["nc"]

    x16 = x.astype(np.float16)
    w_sb = _prep_w(W)
    a_slabs = _prep_adjacency(edge_index)
    b_in = np.ascontiguousarray(b.reshape(1, OUT))
    msel = np.zeros((128, OUT), dtype=np.float32)
    for cg in range(4):
        for j in range(OUT):
            msel[32 * cg + j, j] = 1.0
    in_maps = [
        {"xt": _prep_x_shard(x16, c), "w": w_sb, "at": a_slabs[c], "b": b_in,
         "msel": msel}
        for c in range(NCORES)
    ]

    res = bass_utils.run_bass_kernel_spmd(
        nc, in_maps, core_ids=list(range(NCORES)), trace=_trace,
    )
    _STATE["last_result"] = res

    out = np.empty((N, OUT), dtype=np.float32)
    for c in range(NCORES - 1):
        out[c * RPC:(c + 1) * RPC] = res.results[c]["out"]
    split = (NCORES - 1) * RPC
    out[split:] = res.results[NCORES - 1]["out"][split - _row_start(NCORES - 1):]
    return out


# revision 18
# speedup vs baseline: 1.3464x; 1.3210x over previous
"""GCN one-layer (Cora) on 8 TRN2 NeuronCores.

out = log_softmax(Ahat @ (x @ W) + b), Ahat = D^-1/2 (A + I) D^-1/2 (deg over dst).

Strategy (hardcoded for x[2708,143300] f32, W[143300,7], b[7], edge_index[2,10556]):
  - Row-shard x across 8 cores (340 rows each; core 7 overlaps core 6 by 12 rows
    so every core gets the same shape).  Host casts to fp16 and pre-transposes
    each shard into K-major tiles [1120, 128, 340] so the device GEMM needs no
    on-chip transpose and streams half the bytes (the problem is memory-bound).
  - Per core: h_shard^T = sum_t W[t].T @ xT[t]  (fp16 matmuls, fp32 PSUM).
    Each matmul only uses 7 of 128 PE columns, so 4 K-chunks are packed into
    the 4 column groups via tile_position and run CONCURRENTLY in the array —
    keeps TensorE well below the DMA stream even at the cold 1.2 GHz clock
    (HAM never warms for this shape).  End-to-end output absmax ~9e-4,
    dominated by fp16 rounding of x.
  - The 8 partition-group partials are reduced and transposed in one step by
    a matmul against a 0/1 selector matrix, then the [340,7] h shards are
    AllGathered across the chip.
  - Aggregate with a dense, host-built normalized adjacency (as lhsT tiles,
    fp32), add b via a K=1 matmul with a ones row, log_softmax on device.
  - Host stitches the 8 [340,7] outputs back to [2708,7].
"""

import numpy as np

import concourse.bass as bass
import concourse.mybir as mybir
import concourse.tile as tile
from concourse import bacc
from concourse import bass_utils

N = 2708
K = 143300
OUT = 7
NCORES = 8
RPC = 340                 # rows per core (8*340 = 2720 >= 2708)
KT = 1120                 # 128-wide K chunks (1119 full + 68 remainder, zero padded)
KTAIL = K - 1119 * 128    # 68
CPD = 32                  # K chunks per DMA batch (2.78 MB fp16 per dma_start)
ST = 22                   # s-chunks for aggregation (2720 = 21*128 + 32)
SLAST = NCORES * RPC - 21 * 128  # 32
MTILES = [(0, 128), (128, 128), (256, RPC - 256)]  # out-row tiles per core

f32 = mybir.dt.float32
f16 = mybir.dt.float16

_STATE = {}


def _row_start(c):
    return c * RPC if c < NCORES - 1 else N - RPC


def _build_nc():
    nc = bacc.Bacc(None, target_bir_lowering=False)
    nbatch = KT // CPD
    # batch-major, partition-major layout: one contiguous 21.8 KB run per
    # partition per dma_start (680 B runs crater DMA-engine throughput)
    xt_d = nc.dram_tensor("xt", [nbatch, 128, CPD * RPC], f16, kind="ExternalInput")
    w_d = nc.dram_tensor("w", [128, KT * OUT], f16, kind="ExternalInput")
    a_d = nc.dram_tensor("at", [128, ST * RPC], f32, kind="ExternalInput")
    b_d = nc.dram_tensor("b", [1, OUT], f32, kind="ExternalInput")
    m_d = nc.dram_tensor("msel", [128, OUT], f32, kind="ExternalInput")
    o_d = nc.dram_tensor("out", [RPC, OUT], f32, kind="ExternalOutput")

    with tile.TileContext(nc) as tc:
        with (
            tc.tile_pool(name="wpool", bufs=1) as wpool,
            tc.tile_pool(name="xpool", bufs=3) as xpool,
            tc.tile_pool(name="spool", bufs=1) as spool,
            tc.tile_pool(name="gps", bufs=1, space="PSUM") as gps,
            tc.tile_pool(name="ps2", bufs=2, space="PSUM") as ps2,
            tc.tile_pool(name="dram", bufs=1, space="DRAM") as dram,
        ):
            w_sb = wpool.tile([128, KT * OUT], f16)
            nc.sync.dma_start(w_sb[:], w_d[:])

            # ---- main GEMM: hT[7, RPC] += W[t].T @ xT[t] over 1120 K chunks,
            # 4 chunks packed into the 4 PE column groups (concurrent), two
            # PSUM banks alternating by group.
            ngroups = KT // 4
            hT_pa = gps.tile([128, RPC], f32, tag="pa")
            hT_pb = gps.tile([128, RPC], f32, tag="pb")
            banks = [hT_pa, hT_pb]
            for bi in range(nbatch):
                xt_sb = xpool.tile([128, CPD, RPC], f16, tag="x")
                dma_eng = nc.sync if bi % 2 == 0 else nc.scalar
                dma_eng.dma_start(
                    xt_sb[:].rearrange("p t r -> p (t r)"),
                    xt_d.ap()[bi],
                )
                for ci in range(CPD):
                    t = bi * CPD + ci
                    g, c = t // 4, t % 4
                    nc.tensor.matmul(
                        banks[g % 2][32 * c:32 * c + OUT, :],
                        w_sb[:, t * OUT:(t + 1) * OUT],
                        xt_sb[:, ci, :],
                        tile_position=(0, 32 * c),
                        start=(g < 2),
                        stop=(g >= ngroups - 2),
                    )

            # tail-only operands: loaded behind the x stream on the same queue
            a_sb = wpool.tile([128, ST * RPC], f32)
            nc.sync.dma_start(a_sb[:], a_d[:])
            b_sb = wpool.tile([1, OUT], f32)
            nc.sync.dma_start(b_sb[:], b_d[:])
            m_sb = wpool.tile([128, OUT], f32)
            nc.sync.dma_start(m_sb[:], m_d[:])
            ones_sb = wpool.tile([1, 128], f32)
            nc.vector.memset(ones_sb[:], 1.0)

            # ---- reduce the 8 partition-group partials and transpose to
            # h[RPC, 7] in one step: h[r, j] = sum_bank sum_c psum[32c+j, r],
            # via matmuls against the 0/1 selector msel[32c+j, j] = 1.
            hTa = spool.tile([128, RPC], f32, tag="hta")
            nc.vector.tensor_copy(hTa[:], hT_pa[:])
            hTb = spool.tile([128, RPC], f32, tag="htb")
            nc.vector.tensor_copy(hTb[:], hT_pb[:])
            KSEL = 32 * 3 + OUT  # 103 partitions cover all four groups
            cc_in = dram.tile([RPC, OUT], f32)
            cc_out = dram.tile([NCORES * RPC, OUT], f32)
            for m0, mw in MTILES:
                h_ps = ps2.tile([128, OUT], f32, tag="tp")
                nc.tensor.matmul(h_ps[:mw, :], hTa[:KSEL, m0:m0 + mw],
                                 m_sb[:KSEL, :], start=True, stop=False)
                nc.tensor.matmul(h_ps[:mw, :], hTb[:KSEL, m0:m0 + mw],
                                 m_sb[:KSEL, :], start=False, stop=True)
                hsh_sb = spool.tile([128, OUT], f32, tag="hsh")
                nc.vector.tensor_copy(hsh_sb[:mw, :], h_ps[:mw, :])
                nc.sync.dma_start(cc_in[m0:m0 + mw, :], hsh_sb[:mw, :])
            nc.gpsimd.collective_compute(
                "AllGather",
                mybir.AluOpType.bypass,
                replica_groups=[list(range(NCORES))],
                ins=[cc_in.opt()],
                outs=[cc_out.opt()],
            )

            hall_sb = spool.tile([128, ST, OUT], f32)
            nc.sync.dma_start(
                hall_sb[:, :21, :],
                cc_out[:21 * 128, :].rearrange("(c p) j -> p c j", p=128),
            )
            nc.sync.dma_start(hall_sb[:SLAST, 21, :], cc_out[21 * 128:, :])

            # ---- aggregation + bias + log_softmax per 128-row out tile
            for m0, mw in MTILES:
                o_ps = ps2.tile([128, OUT], f32, tag="agg")
                for c in range(ST):
                    kc = 128 if c < 21 else SLAST
                    nc.tensor.matmul(
                        o_ps[:mw, :],
                        a_sb[:kc, c * RPC + m0:c * RPC + m0 + mw],
                        hall_sb[:kc, c, :],
                        start=(c == 0),
                        stop=False,
                    )
                nc.tensor.matmul(
                    o_ps[:mw, :], ones_sb[:1, :mw], b_sb[:1, :],
                    start=False, stop=True,
                )
                mx = spool.tile([128, 1], f32, tag="mx")
                nc.vector.reduce_max(mx[:mw], o_ps[:mw, :], axis=mybir.AxisListType.X)
                tt = spool.tile([128, OUT], f32, tag="tt")
                nc.vector.tensor_scalar_sub(tt[:mw, :], o_ps[:mw, :], mx[:mw])
                ee = spool.tile([128, OUT], f32, tag="ee")
                ss = spool.tile([128, 1], f32, tag="ss")
                nc.scalar.activation(
                    ee[:mw, :], tt[:mw, :], mybir.ActivationFunctionType.Exp,
                    accum_out=ss[:mw],
                )
                ls = spool.tile([128, 1], f32, tag="ls")
                nc.scalar.activation(
                    ls[:mw], ss[:mw], mybir.ActivationFunctionType.Ln,
                )
                oo = spool.tile([128, OUT], f32, tag="oo")
                nc.vector.tensor_scalar_sub(oo[:mw, :], tt[:mw, :], ls[:mw])
                nc.sync.dma_start(o_d.ap()[m0:m0 + mw, :], oo[:mw, :])

    nc.compile()
    return nc


def _prep_x_shard(x16, c):
    """fp16 [RPC, K] row shard -> [nbatch, 128, CPD, RPC] batch-major tiles.

    out[b, p, t, r] = x[r0+r, (b*CPD+t)*128 + p], zero-padded past K."""
    nbatch = KT // CPD
    r0 = _row_start(c)
    xc = x16[r0:r0 + RPC]
    out = np.empty((nbatch, 128, CPD, RPC), dtype=np.float16)
    span = CPD * 128
    for b in range(nbatch - 1):
        blk = xc[:, b * span:(b + 1) * span]
        out[b] = blk.reshape(RPC, CPD, 128).transpose(2, 1, 0)
    blk = np.zeros((RPC, span), dtype=np.float16)
    blk[:, :K - (nbatch - 1) * span] = xc[:, (nbatch - 1) * span:]
    out[nbatch - 1] = blk.reshape(RPC, CPD, 128).transpose(2, 1, 0)
    return out.reshape(nbatch, 128, CPD * RPC)


def _prep_adjacency(edge_index):
    """Dense Ahat^T tiles per core, laid out as [128, ST*RPC] lhsT slabs."""
    src = np.asarray(edge_index[0], dtype=np.int64)
    dst = np.asarray(edge_index[1], dtype=np.int64)
    deg = np.bincount(dst, minlength=N).astype(np.float64) + 1.0
    dinv = (1.0 / np.sqrt(deg)).astype(np.float32).astype(np.float64)
    A = np.zeros((N, N), dtype=np.float64)
    np.add.at(A, (dst, src), dinv[src] * dinv[dst])
    A[np.arange(N), np.arange(N)] += dinv * dinv
    A = A.astype(np.float32)

    # remap columns to the AllGather layout [NCORES*RPC]: slots 2380..2391 are
    # core 7's copies of rows 2368..2379 (already present) -> zeroed
    split = (NCORES - 1) * RPC                      # 2380
    dup = split - _row_start(NCORES - 1)            # 12
    A_ag = np.zeros((N, NCORES * RPC), dtype=np.float32)
    A_ag[:, :split] = A[:, :split]
    A_ag[:, split + dup:] = A[:, split:]

    slabs = []
    for c in range(NCORES):
        r0 = _row_start(c)
        at = np.zeros((ST * 128, RPC), dtype=np.float32)
        at[:NCORES * RPC] = A_ag[r0:r0 + RPC].T
        slabs.append(
            np.ascontiguousarray(
                at.reshape(ST, 128, RPC).transpose(1, 0, 2).reshape(128, ST * RPC)
            )
        )
    return slabs


def _prep_w(W):
    w_pad = np.zeros((KT * 128, OUT), dtype=np.float16)
    w_pad[:K] = W.astype(np.float16)
    return np.ascontiguousarray(
        w_pad.reshape(KT, 128, OUT).transpose(1, 0, 2).reshape(128, KT * OUT)
    )


def kernel(x, W, b, edge_index, _trace=False):
    x = np.asarray(x, dtype=np.float32)
    W = np.asarray(W, dtype=np.float32)
    b = np.asarray(b, dtype=np.float32)

    if "nc" not in _STATE:
        _STATE["nc"] = _build_nc()
    nc = _STATE["nc"]

    x16 = x.astype(np.float16)
    w_sb = _prep_w(W)
    a_slabs = _prep_adjacency(edge_index)
    b_in = np.ascontiguousarray(b.reshape(1, OUT))
    msel = np.zeros((128, OUT), dtype=np.float32)
    for cg in range(4):
        for j in range(OUT):
            msel[32 * cg + j, j] = 1.0
    in_maps = [
        {"xt": _prep_x_shard(x16, c), "w": w_sb, "at": a_slabs[c], "b": b_in,
         "msel": msel}
        for c in range(NCORES)
    ]

    res = bass_utils.run_bass_kernel_spmd(
        nc, in_maps, core_ids=list(range(NCORES)), trace=_trace,
    )
    _STATE["last_result"] = res

    out = np.empty((N, OUT), dtype=np.float32)
    for c in range(NCORES - 1):
        out[c * RPC:(c + 1) * RPC] = res.results[c]["out"]
    split = (NCORES - 1) * RPC
    out[split:] = res.results[NCORES - 1]["out"][split - _row_start(NCORES - 1):]
    return out


# revision 23
# speedup vs baseline: 1.3986x; 1.0387x over previous
"""GCN one-layer (Cora) on 8 TRN2 NeuronCores.

out = log_softmax(Ahat @ (x @ W) + b), Ahat = D^-1/2 (A + I) D^-1/2 (deg over dst).

Strategy (hardcoded for x[2708,143300] f32, W[143300,7], b[7], edge_index[2,10556]):
  - Row-shard x across 8 cores (340 rows each; core 7 overlaps core 6 by 12 rows
    so every core gets the same shape).  Host casts to fp16 and pre-transposes
    each shard into K-major tiles [1120, 128, 340] so the device GEMM needs no
    on-chip transpose and streams half the bytes (the problem is memory-bound).
  - Per core: h_shard^T = sum_t W[t].T @ xT[t]  (fp16 matmuls, fp32 PSUM).
    Each matmul only uses 7 of 128 PE columns, so 4 K-chunks are packed into
    the 4 column groups via tile_position and run CONCURRENTLY in the array —
    keeps TensorE well below the DMA stream even at the cold 1.2 GHz clock
    (HAM never warms for this shape).  End-to-end output absmax ~9e-4,
    dominated by fp16 rounding of x.
  - The 8 partition-group partials are reduced and transposed in one step by
    a matmul against a 0/1 selector matrix, then the [340,7] h shards are
    AllGathered across the chip.
  - Aggregate with a dense, host-built normalized adjacency (as lhsT tiles,
    fp32), add b via a K=1 matmul with a ones row, log_softmax on device.
  - Host stitches the 8 [340,7] outputs back to [2708,7].
"""

import numpy as np

import concourse.bass as bass
import concourse.mybir as mybir
import concourse.tile as tile
from concourse import bacc
from concourse import bass_utils

N = 2708
K = 143300
OUT = 7
NCORES = 8
RPC = 340                 # rows per core (8*340 = 2720 >= 2708)
KT = 1120                 # 128-wide K chunks (1119 full + 68 remainder, zero padded)
KTAIL = K - 1119 * 128    # 68
CPD = 56                  # K chunks per DMA batch (4.87 MB fp16 per dma_start)
ST = 22                   # s-chunks for aggregation (2720 = 21*128 + 32)
SLAST = NCORES * RPC - 21 * 128  # 32
MTILES = [(0, 128), (128, 128), (256, RPC - 256)]  # out-row tiles per core

f32 = mybir.dt.float32
f16 = mybir.dt.float16

_STATE = {}


def _row_start(c):
    return c * RPC if c < NCORES - 1 else N - RPC


def _build_nc():
    nc = bacc.Bacc(None, target_bir_lowering=False)
    nbatch = KT // CPD
    # batch-major, partition-major layout: one contiguous 21.8 KB run per
    # partition per dma_start (680 B runs crater DMA-engine throughput)
    xt_d = nc.dram_tensor("xt", [nbatch, 128, CPD * RPC], f16, kind="ExternalInput")
    w_d = nc.dram_tensor("w", [128, KT * OUT], f16, kind="ExternalInput")
    a_d = nc.dram_tensor("at", [128, ST * RPC], f32, kind="ExternalInput")
    b_d = nc.dram_tensor("b", [1, OUT], f32, kind="ExternalInput")
    m_d = nc.dram_tensor("msel", [128, OUT], f32, kind="ExternalInput")
    o_d = nc.dram_tensor("out", [RPC, OUT], f32, kind="ExternalOutput")

    with tile.TileContext(nc) as tc:
        with (
            tc.tile_pool(name="wpool", bufs=1) as wpool,
            tc.tile_pool(name="xpool", bufs=3) as xpool,
            tc.tile_pool(name="spool", bufs=3) as spool,
            tc.tile_pool(name="gps", bufs=1, space="PSUM") as gps,
            tc.tile_pool(name="ps2", bufs=2, space="PSUM") as ps2,
            tc.tile_pool(name="ps3", bufs=1, space="PSUM") as ps3,
            tc.tile_pool(name="dram", bufs=1, space="DRAM") as dram,
        ):
            # W on the scalar ring so x batch 0 (sync ring) starts in parallel
            w_sb = wpool.tile([128, KT * OUT], f16)
            nc.scalar.dma_start(w_sb[:], w_d[:])

            # pre-warm the Exp/Ln activation tables while the GEMM streams
            warm = wpool.tile([1, 8], f32)
            nc.vector.memset(warm[:], 1.0)
            nc.scalar.activation(warm[:], warm[:], mybir.ActivationFunctionType.Exp)
            nc.scalar.activation(warm[:], warm[:], mybir.ActivationFunctionType.Ln)

            # ---- main GEMM: hT[7, RPC] += W[t].T @ xT[t] over 1120 K chunks,
            # 4 chunks packed into the 4 PE column groups (concurrent), two
            # PSUM banks alternating by group.
            ngroups = KT // 4
            hT_pa = gps.tile([128, RPC], f32, tag="pa")
            hT_pb = gps.tile([128, RPC], f32, tag="pb")
            banks = [hT_pa, hT_pb]
            for bi in range(nbatch):
                xt_sb = xpool.tile([128, CPD, RPC], f16, tag="x")
                dma_eng = nc.sync if bi % 2 == 0 else nc.scalar
                dma_eng.dma_start(
                    xt_sb[:].rearrange("p t r -> p (t r)"),
                    xt_d.ap()[bi],
                )
                for ci in range(CPD):
                    t = bi * CPD + ci
                    g, c = t // 4, t % 4
                    nc.tensor.matmul(
                        banks[g % 2][32 * c:32 * c + OUT, :],
                        w_sb[:, t * OUT:(t + 1) * OUT],
                        xt_sb[:, ci, :],
                        tile_position=(0, 32 * c),
                        start=(g < 2),
                        stop=(g >= ngroups - 2),
                    )

            # tail-only operands behind the x stream; small ones first so they
            # do not queue behind the 3.8 MB adjacency load
            m_sb = wpool.tile([128, OUT], f32)
            nc.sync.dma_start(m_sb[:], m_d[:])
            b_sb = wpool.tile([1, OUT], f32)
            nc.sync.dma_start(b_sb[:], b_d[:])
            a_sb = wpool.tile([128, ST * RPC], f32)
            nc.sync.dma_start(a_sb[:], a_d[:])
            ones_sb = wpool.tile([1, 128], f32)
            nc.vector.memset(ones_sb[:], 1.0)

            # ---- reduce the 8 partition-group partials and transpose to
            # h[RPC, 7] in one step: h[r, j] = sum_bank sum_c psum[32c+j, r],
            # via matmuls against the 0/1 selector msel[32c+j, j] = 1.
            hTa = spool.tile([128, RPC], f32, tag="hta")
            nc.vector.tensor_copy(hTa[:], hT_pa[:])
            hTb = spool.tile([128, RPC], f32, tag="htb")
            nc.vector.tensor_copy(hTb[:], hT_pb[:])
            KSEL = 32 * 3 + OUT  # 103 partitions cover all four groups
            cc_in = dram.tile([RPC, OUT], f32)
            cc_out = dram.tile([NCORES * RPC, OUT], f32)
            for m0, mw in MTILES:
                h_ps = ps2.tile([128, OUT], f32, tag="tp")
                nc.tensor.matmul(h_ps[:mw, :], hTa[:KSEL, m0:m0 + mw],
                                 m_sb[:KSEL, :], start=True, stop=False)
                nc.tensor.matmul(h_ps[:mw, :], hTb[:KSEL, m0:m0 + mw],
                                 m_sb[:KSEL, :], start=False, stop=True)
                hsh_sb = spool.tile([128, OUT], f32, tag="hsh")
                nc.vector.tensor_copy(hsh_sb[:mw, :], h_ps[:mw, :])
                nc.sync.dma_start(cc_in[m0:m0 + mw, :], hsh_sb[:mw, :])
            nc.gpsimd.collective_compute(
                "AllGather",
                mybir.AluOpType.bypass,
                replica_groups=[list(range(NCORES))],
                ins=[cc_in.opt()],
                outs=[cc_out.opt()],
            )

            hall_sb = spool.tile([128, ST, OUT], f32)
            nc.sync.dma_start(
                hall_sb[:, :21, :],
                cc_out[:21 * 128, :].rearrange("(c p) j -> p c j", p=128),
            )
            nc.sync.dma_start(hall_sb[:SLAST, 21, :], cc_out[21 * 128:, :])

            # ---- aggregation + bias + log_softmax, staged so the Exp/Ln
            # activations batch per function (ACT table reloads cost 1.3 us)
            o_pss, tts, sss, lss = [], [], [], []
            for mi, (m0, mw) in enumerate(MTILES):
                o_ps = ps3.tile([128, OUT], f32, tag=f"agg{mi}")
                for c in range(ST):
                    kc = 128 if c < 21 else SLAST
                    nc.tensor.matmul(
                        o_ps[:mw, :],
                        a_sb[:kc, c * RPC + m0:c * RPC + m0 + mw],
                        hall_sb[:kc, c, :],
                        start=(c == 0),
                        stop=False,
                    )
                nc.tensor.matmul(
                    o_ps[:mw, :], ones_sb[:1, :mw], b_sb[:1, :],
                    start=False, stop=True,
                )
                o_pss.append(o_ps)
            for mi, (m0, mw) in enumerate(MTILES):
                mx = spool.tile([128, 1], f32, tag=f"mx{mi}")
                nc.vector.reduce_max(mx[:mw], o_pss[mi][:mw, :],
                                     axis=mybir.AxisListType.X)
                tt = spool.tile([128, OUT], f32, tag=f"tt{mi}")
                nc.vector.tensor_scalar_sub(tt[:mw, :], o_pss[mi][:mw, :], mx[:mw])
                tts.append(tt)
            for mi, (m0, mw) in enumerate(MTILES):
                ee = spool.tile([128, OUT], f32, tag=f"ee{mi}")
                ss = spool.tile([128, 1], f32, tag=f"ss{mi}")
                nc.scalar.activation(
                    ee[:mw, :], tts[mi][:mw, :], mybir.ActivationFunctionType.Exp,
                    accum_out=ss[:mw],
                )
                sss.append(ss)
            for mi, (m0, mw) in enumerate(MTILES):
                ls = spool.tile([128, 1], f32, tag=f"ls{mi}")
                nc.scalar.activation(ls[:mw], sss[mi][:mw],
                                     mybir.ActivationFunctionType.Ln)
                lss.append(ls)
            for mi, (m0, mw) in enumerate(MTILES):
                oo = spool.tile([128, OUT], f32, tag=f"oo{mi}")
                nc.vector.tensor_scalar_sub(oo[:mw, :], tts[mi][:mw, :],
                                            lss[mi][:mw])
                nc.sync.dma_start(o_d.ap()[m0:m0 + mw, :], oo[:mw, :])

    nc.compile()
    return nc


def _prep_x_shard(x16, c):
    """fp16 [RPC, K] row shard -> [nbatch, 128, CPD, RPC] batch-major tiles.

    out[b, p, t, r] = x[r0+r, (b*CPD+t)*128 + p], zero-padded past K."""
    nbatch = KT // CPD
    r0 = _row_start(c)
    xc = x16[r0:r0 + RPC]
    out = np.empty((nbatch, 128, CPD, RPC), dtype=np.float16)
    span = CPD * 128
    for b in range(nbatch - 1):
        blk = xc[:, b * span:(b + 1) * span]
        out[b] = blk.reshape(RPC, CPD, 128).transpose(2, 1, 0)
    blk = np.zeros((RPC, span), dtype=np.float16)
    blk[:, :K - (nbatch - 1) * span] = xc[:, (nbatch - 1) * span:]
    out[nbatch - 1] = blk.reshape(RPC, CPD, 128).transpose(2, 1, 0)
    return out.reshape(nbatch, 128, CPD * RPC)


def _prep_adjacency(edge_index):
    """Dense Ahat^T tiles per core, laid out as [128, ST*RPC] lhsT slabs."""
    src = np.asarray(edge_index[0], dtype=np.int64)
    dst = np.asarray(edge_index[1], dtype=np.int64)
    deg = np.bincount(dst, minlength=N).astype(np.float64) + 1.0
    dinv = (1.0 / np.sqrt(deg)).astype(np.float32).astype(np.float64)
    A = np.zeros((N, N), dtype=np.float64)
    np.add.at(A, (dst, src), dinv[src] * dinv[dst])
    A[np.arange(N), np.arange(N)] += dinv * dinv
    A = A.astype(np.float32)

    # remap columns to the AllGather layout [NCORES*RPC]: slots 2380..2391 are
    # core 7's copies of rows 2368..2379 (already present) -> zeroed
    split = (NCORES - 1) * RPC                      # 2380
    dup = split - _row_start(NCORES - 1)            # 12
    A_ag = np.zeros((N, NCORES * RPC), dtype=np.float32)
    A_ag[:, :split] = A[:, :split]
    A_ag[:, split + dup:] = A[:, split:]

    slabs = []
    for c in range(NCORES):
        r0 = _row_start(c)
        at = np.zeros((ST * 128, RPC), dtype=np.float32)
        at[:NCORES * RPC] = A_ag[r0:r0 + RPC].T
        slabs.append(
            np.ascontiguousarray(
                at.reshape(ST, 128, RPC).transpose(1, 0, 2).reshape(128, ST * RPC)
            )
        )
    return slabs


def _prep_w(W):
    w_pad = np.zeros((KT * 128, OUT), dtype=np.float16)
    w_pad[:K] = W.astype(np.float16)
    return np.ascontiguousarray(
        w_pad.reshape(KT, 128, OUT).transpose(1, 0, 2).reshape(128, KT * OUT)
    )


def kernel(x, W, b, edge_index, _trace=False):
    x = np.asarray(x, dtype=np.float32)
    W = np.asarray(W, dtype=np.float32)
    b = np.asarray(b, dtype=np.float32)

    if "nc" not in _STATE:
        _STATE["nc"] = _build_nc()
    nc = _STATE["nc"]

    x16 = x.astype(np.float16)
    w_sb = _prep_w(W)
    a_slabs = _prep_adjacency(edge_index)
    b_in = np.ascontiguousarray(b.reshape(1, OUT))
    msel = np.zeros((128, OUT), dtype=np.float32)
    for cg in range(4):
        for j in range(OUT):
            msel[32 * cg + j, j] = 1.0
    in_maps = [
        {"xt": _prep_x_shard(x16, c), "w": w_sb, "at": a_slabs[c], "b": b_in,
         "msel": msel}
        for c in range(NCORES)
    ]

    res = bass_utils.run_bass_kernel_spmd(
        nc, in_maps, core_ids=list(range(NCORES)), trace=_trace,
    )
    _STATE["last_result"] = res

    out = np.empty((N, OUT), dtype=np.float32)
    for c in range(NCORES - 1):
        out[c * RPC:(c + 1) * RPC] = res.results[c]["out"]
    split = (NCORES - 1) * RPC
    out[split:] = res.results[NCORES - 1]["out"][split - _row_start(NCORES - 1):]
    return out
